# revision 18
# baseline (speedup 1.0000x reference)
"""Hypergraph attention (GAT-style) kernel for 8 Trainium2 NeuronCores.

Strategy (self-contained, hardcoded for the nn_Model_13374528160105 shapes):
  - Host: sort edges by hyperedge id (dst), partition hyperedges into 8
    contiguous-by-id, edge-balanced core shards; within a core, bin-pack
    hyperedges into NGROUP groups of <=128 segments / <=TPG*128 edges.
  - Run 1 (device): per-node score table s_all = n_features @ [A_dst|A_src]
    computed from a transposed copy of the feature table, node-sharded.
  - Run 2 (device, per core): for each group, gather per-edge rows of
    cat_table = [n_features | s_all | 1] by src (indirect DMA), build one-hot
    (edge x segment) tiles from in-group dst ids, and do every segment
    reduction / expansion as a TensorEngine matmul against the one-hot (or its
    PE transpose).  Segment softmax skips the max-subtraction (scores are
    O(10), exp is safe in fp32; the reference's eps makes the results equal to
    ~1e-16 relative).
  - Host: scatter per-core local segment rows / per-slot alpha rows back to
    global order.
All float math except the tiny fused-weight precompute (O(D^2)) happens on
device.
"""

import os
import sys
import types

for _p in ("/opt/trn_rl_repo", "/root/.axon_site/_ro/trn_rl_repo"):
    if os.path.isdir(_p) and _p not in sys.path:
        sys.path.append(_p)

import numpy as np

import concourse.bass as bass
import concourse.tile as tile
from concourse import bacc, mybir
from concourse.bass_utils import run_bass_kernel_spmd
from concourse.masks import make_identity

# problem shapes (hardcoded per contract)
N_NODES = 100000
N_HIGH = 20000
N_EDGES = 200000
D = 128
HEADS = 4
EPS = 1e-16

N_CORES = 8
NGROUP = 21          # segment groups per core (<=128 segments each)
TPG = 10             # edge tiles per group (128 edges each)
SEG_CAP = NGROUP * 128
EDGE_CAP = TPG * 128  # per group
CAT = D + 2 * HEADS + 1  # 137: [features | s_dst(4) | s_src(4) | 1]

_f32 = mybir.dt.float32
_bf16 = mybir.dt.bfloat16
_i32 = mybir.dt.int32

_EXEC_TIMES = []  # exec_time_ns per run when KERNEL_TRACE=1


def _trace_on():
    return os.environ.get("KERNEL_TRACE", "0") == "1"


def _maybe_install_profile_hook():
    if not _trace_on():
        return
    if "antenv.axon_hooks" not in sys.modules:
        mod = types.ModuleType("antenv.axon_hooks")
        hook = [None]
        mod.set_axon_ntff_profile_hook = lambda h: hook.__setitem__(0, h)
        mod.get_axon_ntff_profile_hook = lambda: hook[0]
        sys.modules["antenv.axon_hooks"] = mod
        import antenv

        antenv.axon_hooks = mod
    import antenv.axon_hooks as ah

    if ah.get_axon_ntff_profile_hook() is None:
        if "/root/.axon_site" not in sys.path:
            sys.path.insert(0, "/root/.axon_site")
        from trn_agent_boot.trn_boot import _ntff_profile_via_ctypes

        ah.set_axon_ntff_profile_hook(
            _ntff_profile_via_ctypes("/opt/axon/libaxon_pjrt.so"))


# ---------------------------------------------------------------- run 1 ----

_NC1 = None
NSLICE = N_NODES // N_CORES  # 12500


def _build_run1():
    global _NC1
    if _NC1 is not None:
        return _NC1
    nc = bacc.Bacc("TRN2", target_bir_lowering=False, debug=False,
                   num_devices=N_CORES)
    nfT = nc.dram_tensor("nfT", [D, NSLICE], _f32, kind="ExternalInput").ap()
    a8 = nc.dram_tensor("a8", [D, 2 * HEADS], _f32, kind="ExternalInput").ap()
    out = nc.dram_tensor("sallT", [2 * HEADS, NSLICE], _f32,
                         kind="ExternalOutput").ap()
    CH = 500
    nch = NSLICE // CH
    with tile.TileContext(nc) as tc:
        with tc.tile_pool(name="sb", bufs=1) as pool, \
             tc.tile_pool(name="ps", bufs=4, space="PSUM") as psp, \
             tc.tile_pool(name="w", bufs=1) as wp:
            a8_t = wp.tile([D, 2 * HEADS], _f32)
            nc.sync.dma_start(a8_t[:], a8[:])
            x = pool.tile([D, NSLICE], _f32)
            nc.sync.dma_start(x[:], nfT[:])
            o = pool.tile([2 * HEADS, NSLICE], _f32)
            for c in range(nch):
                lo = c * CH
                ps = psp.tile([2 * HEADS, CH], _f32, tag="ps")
                nc.tensor.matmul(ps[:], lhsT=a8_t[:], rhs=x[:, lo:lo + CH],
                                 start=True, stop=True)
                nc.vector.tensor_copy(o[:, lo:lo + CH], ps[:])
            nc.sync.dma_start(out[:], o[:])
    nc.compile()
    _NC1 = nc
    return nc


# ---------------------------------------------------------------- run 2 ----

_NC2 = None


def _build_run2():
    global _NC2
    if _NC2 is not None:
        return _NC2
    nc = bacc.Bacc("TRN2", target_bir_lowering=False, debug=False,
                   num_devices=N_CORES)
    cat = nc.dram_tensor("cat", [N_NODES, CAT], _bf16, kind="ExternalInput").ap()
    srcg = nc.dram_tensor("srcg", [NGROUP, 128, TPG], _i32,
                          kind="ExternalInput").ap()
    dstig = nc.dram_tensor("dstig", [NGROUP, 128, TPG], _bf16,
                           kind="ExternalInput").ap()
    iota = nc.dram_tensor("iota", [128, 128], _bf16, kind="ExternalInput").ap()
    bias = nc.dram_tensor("bias", [128, D], _f32, kind="ExternalInput").ap()
    mq = nc.dram_tensor("mq", [D, HEADS * D], _bf16, kind="ExternalInput").ap()
    alpha_o = nc.dram_tensor("alpha", [NGROUP, 128, TPG * HEADS], _f32,
                             kind="ExternalOutput").ap()
    hf_o = nc.dram_tensor("hf", [NGROUP * 128, D], _f32,
                          kind="ExternalOutput").ap()

    H4 = TPG * HEADS  # 40

    with tile.TileContext(nc) as tc:
        with tc.tile_pool(name="const", bufs=1) as cp, \
             tc.tile_pool(name="big", bufs=3) as bp, \
             tc.tile_pool(name="sm", bufs=3) as sp, \
             tc.tile_pool(name="ps1", bufs=1, space="PSUM") as ps1, \
             tc.tile_pool(name="psT", bufs=2, space="PSUM") as psT, \
             tc.tile_pool(name="pso", bufs=1, space="PSUM") as pso, \
             tc.tile_pool(name="psg", bufs=3, space="PSUM") as psg:
            iota_t = cp.tile([128, 128], _bf16)
            nc.sync.dma_start(iota_t[:], iota[:])
            bias_t = cp.tile([128, D], _f32)
            nc.sync.dma_start(bias_t[:], bias[:])
            mq_t = cp.tile([D, HEADS * D], _bf16)
            nc.sync.dma_start(mq_t[:], mq[:])
            ident = cp.tile([128, 128], _bf16)
            make_identity(nc, ident[:])

            for g in range(NGROUP):
                idx_g = sp.tile([128, TPG], _i32, tag="idx")
                nc.sync.dma_start(idx_g[:], srcg[g])
                dig_g = sp.tile([128, TPG], _bf16, tag="dig")
                nc.sync.dma_start(dig_g[:], dstig[g])

                nn9 = bp.tile([128, TPG * CAT], _bf16, tag="nn9")
                for k in range(TPG):
                    nc.gpsimd.indirect_dma_start(
                        out=nn9[:, k * CAT:(k + 1) * CAT], out_offset=None,
                        in_=cat[:],
                        in_offset=bass.IndirectOffsetOnAxis(
                            ap=idx_g[:, k:k + 1], axis=0))

                oh = bp.tile([128, TPG * 128], _bf16, tag="oh")
                ohT = bp.tile([128, TPG * 128], _bf16, tag="ohT")
                ps_a = ps1.tile([128, 8], _f32, tag="ps_a")
                for k in range(TPG):
                    ohk = oh[:, k * 128:(k + 1) * 128]
                    nc.vector.tensor_tensor(
                        out=ohk, in0=dig_g[:, k:k + 1].to_broadcast([128, 128]),
                        in1=iota_t[:], op=mybir.AluOpType.is_equal)
                    # segment partial sums of [s_src scores | 1] -> [seg, 5]
                    nc.tensor.matmul(
                        ps_a[:, :5], lhsT=ohk,
                        rhs=nn9[:, k * CAT + D + HEADS:(k + 1) * CAT],
                        start=(k == 0), stop=(k == TPG - 1))
                    pT = psT.tile([128, 128], _bf16, tag="pT")
                    nc.tensor.transpose(out=pT[:], in_=ohk, identity=ident[:])
                    nc.scalar.copy(ohT[:, k * 128:(k + 1) * 128], pT[:])

                # s_high = segsum(src-score) / max(cnt, 1)
                rc = sp.tile([128, 1], _f32, tag="rc")
                nc.vector.tensor_scalar(out=rc[:], in0=ps_a[:, 4:5],
                                        scalar1=1.0, scalar2=None,
                                        op0=mybir.AluOpType.max)
                nc.vector.reciprocal(rc[:], rc[:])
                sh = sp.tile([128, HEADS], _f32, tag="sh")
                nc.vector.tensor_scalar(out=sh[:], in0=ps_a[:, 0:4],
                                        scalar1=rc[:, :1], scalar2=None,
                                        op0=mybir.AluOpType.mult)

                shb = sp.tile([128, HEADS], _bf16, tag="shb")
                nc.vector.tensor_copy(shb[:], sh[:])
                # expand s_high to edges: ohT.T @ sh  -> [edge, 4] per tile
                ps_she = ps1.tile([128, H4], _f32, tag="ps_small")
                for k in range(TPG):
                    nc.tensor.matmul(
                        ps_she[:, k * HEADS:(k + 1) * HEADS],
                        lhsT=ohT[:, k * 128:(k + 1) * 128], rhs=shb[:],
                        start=True, stop=True)

                # raw scores -> exp(leaky_relu(.))
                araw = sp.tile([128, H4], _f32, tag="araw")
                nn9_3 = nn9[:].rearrange("p (k c) -> p k c", k=TPG)
                nc.vector.tensor_tensor(
                    out=araw[:].rearrange("p (k h) -> p k h", k=TPG),
                    in0=nn9_3[:, :, D:D + HEADS], in1=ps_she[:].rearrange(
                        "p (k h) -> p k h", k=TPG),
                    op=mybir.AluOpType.add)
                lr = sp.tile([128, H4], _f32, tag="lr")
                nc.vector.tensor_scalar(out=lr[:], in0=araw[:], scalar1=0.2,
                                        scalar2=None, op0=mybir.AluOpType.mult)
                nc.vector.tensor_tensor(out=lr[:], in0=lr[:], in1=araw[:],
                                        op=mybir.AluOpType.max)
                ea = sp.tile([128, H4], _f32, tag="ea")
                nc.scalar.activation(ea[:], lr[:],
                                     mybir.ActivationFunctionType.Exp)

                eab = sp.tile([128, H4], _bf16, tag="eab")
                nc.vector.tensor_copy(eab[:], ea[:])
                # asum per segment
                ps_as = ps1.tile([128, H4], _f32, tag="ps_small")
                for k in range(TPG):
                    nc.tensor.matmul(ps_as[:, :HEADS],
                                     lhsT=oh[:, k * 128:(k + 1) * 128],
                                     rhs=eab[:, k * HEADS:(k + 1) * HEADS],
                                     start=(k == 0), stop=(k == TPG - 1))
                rec = sp.tile([128, HEADS], _f32, tag="rec")
                nc.vector.tensor_scalar(out=rec[:], in0=ps_as[:, :HEADS], scalar1=EPS,
                                        scalar2=None, op0=mybir.AluOpType.add)
                nc.vector.reciprocal(rec[:], rec[:])

                recb = sp.tile([128, HEADS], _bf16, tag="recb")
                nc.vector.tensor_copy(recb[:], rec[:])
                # expand recip(asum) to edges, alpha = ea * rec[dst]
                ps_rec = ps1.tile([128, H4], _f32, tag="ps_small")
                for k in range(TPG):
                    nc.tensor.matmul(
                        ps_rec[:, k * HEADS:(k + 1) * HEADS],
                        lhsT=ohT[:, k * 128:(k + 1) * 128], rhs=recb[:],
                        start=True, stop=True)
                al = sp.tile([128, H4], _f32, tag="al")
                nc.vector.tensor_tensor(out=al[:], in0=ea[:], in1=ps_rec[:],
                                        op=mybir.AluOpType.mult)
                nc.sync.dma_start(alpha_o[g], al[:])

                # weighted segment sums g_hT[d, s] via scaled one-hots
                ps_g = psg.tile([128, HEADS * 128], _f32, tag="ps_g")
                sc4 = bp.tile([128, HEADS * 128], _bf16, tag="sc4")
                for k in range(TPG):
                    for h in range(HEADS):
                        nc.vector.tensor_scalar(
                            out=sc4[:, h * 128:(h + 1) * 128],
                            in0=oh[:, k * 128:(k + 1) * 128],
                            scalar1=al[:, k * HEADS + h:k * HEADS + h + 1],
                            scalar2=None, op0=mybir.AluOpType.mult)
                    nc.tensor.matmul(ps_g[:], lhsT=nn9[:, k * CAT:k * CAT + D],
                                     rhs=sc4[:], start=(k == 0),
                                     stop=(k == TPG - 1))
                gt = bp.tile([128, HEADS * 128], _bf16, tag="gt")
                nc.scalar.copy(gt[:, :256], ps_g[:, :256])
                nc.vector.tensor_copy(gt[:, 256:], ps_g[:, 256:])

                # h_out[s, d'] = sum_h g_hT.T @ Mq_h  (+ bias)
                ps_o = pso.tile([128, D], _f32, tag="ps_o")
                for h in range(HEADS):
                    nc.tensor.matmul(ps_o[:], lhsT=gt[:, h * 128:(h + 1) * 128],
                                     rhs=mq_t[:, h * D:(h + 1) * D],
                                     start=(h == 0), stop=(h == HEADS - 1))
                hf = sp.tile([128, D], _f32, tag="hf")
                nc.vector.tensor_tensor(out=hf[:], in0=ps_o[:], in1=bias_t[:],
                                        op=mybir.AluOpType.add)
                nc.sync.dma_start(hf_o[g * 128:(g + 1) * 128, :], hf[:])
    nc.compile()
    _NC2 = nc
    return nc


# ------------------------------------------------------------ host prep ----

def _prep_edges(dst):
    """Partition sorted-by-dst edges into cores/groups/tiles.

    Returns per-core dicts with device index arrays and unshard maps."""
    E = dst.shape[0]
    counts = np.bincount(dst, minlength=N_HIGH).astype(np.int64)
    order = np.argsort(dst, kind="stable")
    seg_start = np.zeros(N_HIGH + 1, np.int64)
    np.cumsum(counts, out=seg_start[1:])

    # split hyperedge ids into 8 shards balanced by edge count
    target = E / N_CORES
    bounds = [0]
    for c in range(1, N_CORES):
        bounds.append(int(np.searchsorted(seg_start, round(target * c))))
    bounds.append(N_HIGH)

    cores = []
    for c in range(N_CORES):
        s_lo, s_hi = bounds[c], bounds[c + 1]
        segs = np.arange(s_lo, s_hi)
        cnt = counts[s_lo:s_hi]
        assert len(segs) <= SEG_CAP, (c, len(segs))
        assert cnt.max(initial=0) <= EDGE_CAP

        # bin-pack segments into NGROUP groups (<=128 segs, <=EDGE_CAP edges)
        g_load = np.zeros(NGROUP, np.int64)
        g_segs = [[] for _ in range(NGROUP)]
        for s in segs[np.argsort(-cnt, kind="stable")]:
            k = counts[s]
            ok = [g for g in range(NGROUP)
                  if len(g_segs[g]) < 128 and g_load[g] + k <= EDGE_CAP]
            assert ok, "bin packing failed; raise TPG/NGROUP"
            g = min(ok, key=lambda g: g_load[g])
            g_segs[g].append(s)
            g_load[g] += k

        edge_orig = np.full((NGROUP, TPG * 128), -1, np.int64)
        gseg = np.full(NGROUP * 128, -1, np.int64)
        for g in range(NGROUP):
            j = 0
            for ig, s in enumerate(g_segs[g]):
                gseg[g * 128 + ig] = s
                e_ids = order[seg_start[s]:seg_start[s + 1]]
                n = len(e_ids)
                edge_orig[g, j:j + n] = e_ids
                j += n
            # slot j -> tile k=j//128, partition p=j%128
        cores.append(dict(g_segs=g_segs, edge_orig=edge_orig,
                          valid=edge_orig >= 0, gseg=gseg))
    return cores, order, seg_start


def _prep(src, dst):
    cores, order, seg_start = _prep_edges(dst)
    counts = np.bincount(dst, minlength=N_HIGH).astype(np.int64)
    for cd in cores:
        eo = cd["edge_orig"]          # [NGROUP, TPG*128] slot-order
        valid = cd["valid"]
        # slot j in group: tile k=j//128, partition p=j%128
        src_slot = np.where(valid, src[np.clip(eo, 0, None)], 0)
        # dstig already filled per-slot during packing -> redo cleanly here
        srcg = src_slot.reshape(NGROUP, TPG, 128).transpose(0, 2, 1)
        cd["srcg"] = np.ascontiguousarray(srcg.astype(np.int32))
        dig_slot = np.full((NGROUP, TPG * 128), 999.0, np.float32)
        # in-group id per slot: recompute from g_segs
        for g, segs_g in enumerate(cd["g_segs"]):
            j = 0
            for ig, s in enumerate(segs_g):
                n = int(counts[s])
                dig_slot[g, j:j + n] = ig
                j += n
        import ml_dtypes
        cd["dstig"] = np.ascontiguousarray(
            dig_slot.reshape(NGROUP, TPG, 128).transpose(0, 2, 1)).astype(
                ml_dtypes.bfloat16)
    return cores


def _run(nc, in_maps):
    _maybe_install_profile_hook()
    res = run_bass_kernel_spmd(nc, in_maps, core_ids=list(range(N_CORES)),
                               trace=_trace_on())
    if res.exec_time_ns is not None:
        _EXEC_TIMES.append(res.exec_time_ns)
    return res.results


def kernel(n_features, W_src, W_dst, att_src, att_dst, W_high, b_high,
           src_idx, dst_idx):
    n_features = np.asarray(n_features, np.float32)
    W_src = np.asarray(W_src, np.float32)
    W_dst = np.asarray(W_dst, np.float32)
    att_src = np.asarray(att_src, np.float32)
    att_dst = np.asarray(att_dst, np.float32)
    W_high = np.asarray(W_high, np.float32)
    b_high = np.asarray(b_high, np.float32)
    src = np.asarray(src_idx).astype(np.int64)
    dst = np.asarray(dst_idx).astype(np.int64)
    E = src.shape[0]

    # fused weights (tiny O(D^2) host math)
    A_dst = np.einsum("dhk,hk->dh", W_src.reshape(D, HEADS, D),
                      att_dst[0]).astype(np.float32)
    A_src = np.einsum("dhk,hk->dh", W_dst.reshape(D, HEADS, D),
                      att_src[0]).astype(np.float32)
    Mq = (0.25 * np.einsum("dhk,ke->dhe", W_src.reshape(D, HEADS, D),
                           W_high)).reshape(D, HEADS * D).astype(np.float32)

    # ---- run 1: per-node scores ----
    nc1 = _build_run1()
    nfT = np.ascontiguousarray(n_features.T)           # [D, N]
    a8 = np.concatenate([A_dst, A_src], axis=1)        # [D, 8]
    in1 = [{"nfT": np.ascontiguousarray(nfT[:, c * NSLICE:(c + 1) * NSLICE]),
            "a8": a8} for c in range(N_CORES)]
    r1 = _run(nc1, in1)
    s_all = np.concatenate([r1[c]["sallT"] for c in range(N_CORES)],
                           axis=1).T                   # [N, 8]

    import ml_dtypes
    cat = np.empty((N_NODES, CAT), ml_dtypes.bfloat16)
    cat[:, :D] = n_features
    cat[:, D:D + 2 * HEADS] = s_all
    cat[:, D + 2 * HEADS] = 1.0

    # ---- host prep of edge structure ----
    cores = _prep(src, dst)

    import ml_dtypes
    iota = np.ascontiguousarray(np.broadcast_to(
        np.arange(128, dtype=np.float32), (128, 128))).astype(ml_dtypes.bfloat16)
    bias = np.ascontiguousarray(np.broadcast_to(b_high, (128, D))).astype(
        np.float32)

    nc2 = _build_run2()
    in2 = [{"cat": cat, "srcg": cd["srcg"], "dstig": cd["dstig"],
            "iota": iota, "bias": bias, "mq": Mq.astype(ml_dtypes.bfloat16)} for cd in cores]
    r2 = _run(nc2, in2)

    # ---- unshard ----
    h_features = np.zeros((N_HIGH, D), np.float32)
    alpha = np.zeros((E, HEADS), np.float32)
    for c, cd in enumerate(cores):
        hf = r2[c]["hf"]                               # [NGROUP*128, D]
        gseg = cd["gseg"]
        m = gseg >= 0
        h_features[gseg[m]] = hf[m]
        al = r2[c]["alpha"].reshape(NGROUP, 128, TPG, HEADS)
        al = al.transpose(0, 2, 1, 3).reshape(NGROUP, TPG * 128, HEADS)
        eo = cd["edge_orig"]
        v = cd["valid"]
        alpha[eo[v]] = al[v]
    return h_features, alpha


# revision 19
# speedup vs baseline: 1.0189x; 1.0189x over previous
"""Hypergraph attention (GAT-style) kernel for 8 Trainium2 NeuronCores.

Strategy (self-contained, hardcoded for the nn_Model_13374528160105 shapes):
  - Host: sort edges by hyperedge id (dst), partition hyperedges into 8
    contiguous-by-id, edge-balanced core shards; within a core, bin-pack
    hyperedges into NGROUP groups of <=128 segments / <=TPG*128 edges.
  - Run 1 (device): per-node score table s_all = n_features @ [A_dst|A_src]
    computed from a transposed copy of the feature table, node-sharded.
  - Run 2 (device, per core): for each group, gather per-edge rows of
    cat_table = [n_features | s_all | 1] by src (indirect DMA), build one-hot
    (edge x segment) tiles from in-group dst ids, and do every segment
    reduction / expansion as a TensorEngine matmul against the one-hot (or its
    PE transpose).  Segment softmax skips the max-subtraction (scores are
    O(10), exp is safe in fp32; the reference's eps makes the results equal to
    ~1e-16 relative).
  - Host: scatter per-core local segment rows / per-slot alpha rows back to
    global order.
All float math except the tiny fused-weight precompute (O(D^2)) happens on
device.
"""

import os
import sys
import types

for _p in ("/opt/trn_rl_repo", "/root/.axon_site/_ro/trn_rl_repo"):
    if os.path.isdir(_p) and _p not in sys.path:
        sys.path.append(_p)

import numpy as np

import concourse.bass as bass
import concourse.tile as tile
from concourse import bacc, mybir
from concourse.bass_utils import run_bass_kernel_spmd
from concourse.masks import make_identity

# problem shapes (hardcoded per contract)
N_NODES = 100000
N_HIGH = 20000
N_EDGES = 200000
D = 128
HEADS = 4
EPS = 1e-16

N_CORES = 8
NGROUP = 21          # segment groups per core (<=128 segments each)
TPG = 10             # edge tiles per group (128 edges each)
SEG_CAP = NGROUP * 128
EDGE_CAP = TPG * 128  # per group
CAT = D + 2 * HEADS + 1  # 137: [features | s_dst(4) | s_src(4) | 1]

_f32 = mybir.dt.float32
_bf16 = mybir.dt.bfloat16
_i32 = mybir.dt.int32

_EXEC_TIMES = []  # exec_time_ns per run when KERNEL_TRACE=1


def _trace_on():
    return os.environ.get("KERNEL_TRACE", "0") == "1"


def _maybe_install_profile_hook():
    if not _trace_on():
        return
    if "antenv.axon_hooks" not in sys.modules:
        mod = types.ModuleType("antenv.axon_hooks")
        hook = [None]
        mod.set_axon_ntff_profile_hook = lambda h: hook.__setitem__(0, h)
        mod.get_axon_ntff_profile_hook = lambda: hook[0]
        sys.modules["antenv.axon_hooks"] = mod
        import antenv

        antenv.axon_hooks = mod
    import antenv.axon_hooks as ah

    if ah.get_axon_ntff_profile_hook() is None:
        if "/root/.axon_site" not in sys.path:
            sys.path.insert(0, "/root/.axon_site")
        from trn_agent_boot.trn_boot import _ntff_profile_via_ctypes

        ah.set_axon_ntff_profile_hook(
            _ntff_profile_via_ctypes("/opt/axon/libaxon_pjrt.so"))


# ---------------------------------------------------------------- run 1 ----

_NC1 = None
NSLICE = N_NODES // N_CORES  # 12500


def _build_run1():
    global _NC1
    if _NC1 is not None:
        return _NC1
    nc = bacc.Bacc("TRN2", target_bir_lowering=False, debug=False,
                   num_devices=N_CORES)
    nfT = nc.dram_tensor("nfT", [D, NSLICE], _f32, kind="ExternalInput").ap()
    a8 = nc.dram_tensor("a8", [D, 2 * HEADS], _f32, kind="ExternalInput").ap()
    out = nc.dram_tensor("sallT", [2 * HEADS, NSLICE], _f32,
                         kind="ExternalOutput").ap()
    CH = 500
    nch = NSLICE // CH
    with tile.TileContext(nc) as tc:
        with tc.tile_pool(name="sb", bufs=1) as pool, \
             tc.tile_pool(name="ps", bufs=4, space="PSUM") as psp, \
             tc.tile_pool(name="w", bufs=1) as wp:
            a8_t = wp.tile([D, 2 * HEADS], _f32)
            nc.sync.dma_start(a8_t[:], a8[:])
            x = pool.tile([D, NSLICE], _f32)
            nc.sync.dma_start(x[:], nfT[:])
            o = pool.tile([2 * HEADS, NSLICE], _f32)
            for c in range(nch):
                lo = c * CH
                ps = psp.tile([2 * HEADS, CH], _f32, tag="ps")
                nc.tensor.matmul(ps[:], lhsT=a8_t[:], rhs=x[:, lo:lo + CH],
                                 start=True, stop=True)
                nc.vector.tensor_copy(o[:, lo:lo + CH], ps[:])
            nc.sync.dma_start(out[:], o[:])
    nc.compile()
    _NC1 = nc
    return nc


# ---------------------------------------------------------------- run 2 ----

_NC2 = None


def _build_run2():
    global _NC2
    if _NC2 is not None:
        return _NC2
    nc = bacc.Bacc("TRN2", target_bir_lowering=False, debug=False,
                   num_devices=N_CORES)
    cat = nc.dram_tensor("cat", [N_NODES, CAT], _bf16, kind="ExternalInput").ap()
    srcg = nc.dram_tensor("srcg", [NGROUP, 128, TPG], _i32,
                          kind="ExternalInput").ap()
    dstig = nc.dram_tensor("dstig", [NGROUP, 128, TPG], _bf16,
                           kind="ExternalInput").ap()
    iota = nc.dram_tensor("iota", [128, 128], _bf16, kind="ExternalInput").ap()
    bias = nc.dram_tensor("bias", [128, D], _f32, kind="ExternalInput").ap()
    mq = nc.dram_tensor("mq", [D, HEADS * D], _bf16, kind="ExternalInput").ap()
    alpha_o = nc.dram_tensor("alpha", [NGROUP, 128, TPG * HEADS], _f32,
                             kind="ExternalOutput").ap()
    hf_o = nc.dram_tensor("hf", [NGROUP * 128, D], _f32,
                          kind="ExternalOutput").ap()

    H4 = TPG * HEADS  # 40

    with tile.TileContext(nc) as tc:
        with tc.tile_pool(name="const", bufs=1) as cp, \
             tc.tile_pool(name="big", bufs=3) as bp, \
             tc.tile_pool(name="sm", bufs=3) as sp, \
             tc.tile_pool(name="ps1", bufs=1, space="PSUM") as ps1, \
             tc.tile_pool(name="psT", bufs=2, space="PSUM") as psT, \
             tc.tile_pool(name="pso", bufs=1, space="PSUM") as pso, \
             tc.tile_pool(name="psg", bufs=2, space="PSUM") as psg:
            iota_t = cp.tile([128, 128], _bf16)
            nc.sync.dma_start(iota_t[:], iota[:])
            bias_t = cp.tile([128, D], _f32)
            nc.sync.dma_start(bias_t[:], bias[:])
            mq_t = cp.tile([D, HEADS * D], _bf16)
            nc.sync.dma_start(mq_t[:], mq[:])
            ident = cp.tile([128, 128], _bf16)
            make_identity(nc, ident[:])

            for g in range(NGROUP):
                idx_g = sp.tile([128, TPG], _i32, tag="idx")
                nc.sync.dma_start(idx_g[:], srcg[g])
                dig_g = sp.tile([128, TPG], _bf16, tag="dig")
                nc.sync.dma_start(dig_g[:], dstig[g])

                nn9 = bp.tile([128, TPG * CAT], _bf16, tag="nn9")
                for k in range(TPG):
                    nc.gpsimd.indirect_dma_start(
                        out=nn9[:, k * CAT:(k + 1) * CAT], out_offset=None,
                        in_=cat[:],
                        in_offset=bass.IndirectOffsetOnAxis(
                            ap=idx_g[:, k:k + 1], axis=0))

                oh = bp.tile([128, TPG * 128], _bf16, tag="oh")
                ohT = bp.tile([128, TPG * 128], _bf16, tag="ohT")
                ps_a = ps1.tile([128, 8], _f32, tag="ps_a")
                for k in range(TPG):
                    ohk = oh[:, k * 128:(k + 1) * 128]
                    nc.vector.tensor_tensor(
                        out=ohk, in0=dig_g[:, k:k + 1].to_broadcast([128, 128]),
                        in1=iota_t[:], op=mybir.AluOpType.is_equal)
                    # segment partial sums of [s_src scores | 1] -> [seg, 5]
                    nc.tensor.matmul(
                        ps_a[:, :5], lhsT=ohk,
                        rhs=nn9[:, k * CAT + D + HEADS:(k + 1) * CAT],
                        start=(k == 0), stop=(k == TPG - 1))
                    pT = psT.tile([128, 128], _bf16, tag="pT")
                    nc.tensor.transpose(out=pT[:], in_=ohk, identity=ident[:])
                    nc.scalar.copy(ohT[:, k * 128:(k + 1) * 128], pT[:])

                # s_high = segsum(src-score) / max(cnt, 1)
                rc = sp.tile([128, 1], _f32, tag="rc")
                nc.vector.tensor_scalar(out=rc[:], in0=ps_a[:, 4:5],
                                        scalar1=1.0, scalar2=None,
                                        op0=mybir.AluOpType.max)
                nc.vector.reciprocal(rc[:], rc[:])
                sh = sp.tile([128, HEADS], _f32, tag="sh")
                nc.vector.tensor_scalar(out=sh[:], in0=ps_a[:, 0:4],
                                        scalar1=rc[:, :1], scalar2=None,
                                        op0=mybir.AluOpType.mult)

                shb = sp.tile([128, HEADS], _bf16, tag="shb")
                nc.vector.tensor_copy(shb[:], sh[:])
                # expand s_high to edges: ohT.T @ sh  -> [edge, 4] per tile
                ps_she = psT.tile([128, H4], _f32, tag="ps_small")
                for k in range(TPG):
                    nc.tensor.matmul(
                        ps_she[:, k * HEADS:(k + 1) * HEADS],
                        lhsT=ohT[:, k * 128:(k + 1) * 128], rhs=shb[:],
                        start=True, stop=True)

                # raw scores -> exp(leaky_relu(.))
                araw = sp.tile([128, H4], _f32, tag="araw")
                nn9_3 = nn9[:].rearrange("p (k c) -> p k c", k=TPG)
                nc.vector.tensor_tensor(
                    out=araw[:].rearrange("p (k h) -> p k h", k=TPG),
                    in0=nn9_3[:, :, D:D + HEADS], in1=ps_she[:].rearrange(
                        "p (k h) -> p k h", k=TPG),
                    op=mybir.AluOpType.add)
                lr = sp.tile([128, H4], _f32, tag="lr")
                nc.vector.tensor_scalar(out=lr[:], in0=araw[:], scalar1=0.2,
                                        scalar2=None, op0=mybir.AluOpType.mult)
                nc.vector.tensor_tensor(out=lr[:], in0=lr[:], in1=araw[:],
                                        op=mybir.AluOpType.max)
                ea = sp.tile([128, H4], _f32, tag="ea")
                nc.scalar.activation(ea[:], lr[:],
                                     mybir.ActivationFunctionType.Exp)

                eab = sp.tile([128, H4], _bf16, tag="eab")
                nc.vector.tensor_copy(eab[:], ea[:])
                # asum per segment
                ps_as = psT.tile([128, H4], _f32, tag="ps_small")
                for k in range(TPG):
                    nc.tensor.matmul(ps_as[:, :HEADS],
                                     lhsT=oh[:, k * 128:(k + 1) * 128],
                                     rhs=eab[:, k * HEADS:(k + 1) * HEADS],
                                     start=(k == 0), stop=(k == TPG - 1))
                rec = sp.tile([128, HEADS], _f32, tag="rec")
                nc.vector.tensor_scalar(out=rec[:], in0=ps_as[:, :HEADS], scalar1=EPS,
                                        scalar2=None, op0=mybir.AluOpType.add)
                nc.vector.reciprocal(rec[:], rec[:])

                recb = sp.tile([128, HEADS], _bf16, tag="recb")
                nc.vector.tensor_copy(recb[:], rec[:])
                # expand recip(asum) to edges, alpha = ea * rec[dst]
                ps_rec = psT.tile([128, H4], _f32, tag="ps_small")
                for k in range(TPG):
                    nc.tensor.matmul(
                        ps_rec[:, k * HEADS:(k + 1) * HEADS],
                        lhsT=ohT[:, k * 128:(k + 1) * 128], rhs=recb[:],
                        start=True, stop=True)
                al = sp.tile([128, H4], _f32, tag="al")
                nc.vector.tensor_tensor(out=al[:], in0=ea[:], in1=ps_rec[:],
                                        op=mybir.AluOpType.mult)
                nc.sync.dma_start(alpha_o[g], al[:])

                # weighted segment sums g_hT[d, s] via scaled one-hots
                ps_g = psg.tile([128, HEADS * 128], _f32, tag="ps_g")
                sc4 = bp.tile([128, HEADS * 128], _bf16, tag="sc4")
                for k in range(TPG):
                    for h in range(HEADS):
                        nc.vector.tensor_scalar(
                            out=sc4[:, h * 128:(h + 1) * 128],
                            in0=oh[:, k * 128:(k + 1) * 128],
                            scalar1=al[:, k * HEADS + h:k * HEADS + h + 1],
                            scalar2=None, op0=mybir.AluOpType.mult)
                    nc.tensor.matmul(ps_g[:], lhsT=nn9[:, k * CAT:k * CAT + D],
                                     rhs=sc4[:], start=(k == 0),
                                     stop=(k == TPG - 1))
                gt = bp.tile([128, HEADS * 128], _bf16, tag="gt")
                nc.scalar.copy(gt[:, :256], ps_g[:, :256])
                nc.vector.tensor_copy(gt[:, 256:], ps_g[:, 256:])

                # h_out[s, d'] = sum_h g_hT.T @ Mq_h  (+ bias)
                ps_o = pso.tile([128, D], _f32, tag="ps_o")
                for h in range(HEADS):
                    nc.tensor.matmul(ps_o[:], lhsT=gt[:, h * 128:(h + 1) * 128],
                                     rhs=mq_t[:, h * D:(h + 1) * D],
                                     start=(h == 0), stop=(h == HEADS - 1))
                hf = sp.tile([128, D], _f32, tag="hf")
                nc.vector.tensor_tensor(out=hf[:], in0=ps_o[:], in1=bias_t[:],
                                        op=mybir.AluOpType.add)
                nc.sync.dma_start(hf_o[g * 128:(g + 1) * 128, :], hf[:])
    nc.compile()
    _NC2 = nc
    return nc


# ------------------------------------------------------------ host prep ----

def _prep_edges(dst):
    """Partition sorted-by-dst edges into cores/groups/tiles.

    Returns per-core dicts with device index arrays and unshard maps."""
    E = dst.shape[0]
    counts = np.bincount(dst, minlength=N_HIGH).astype(np.int64)
    order = np.argsort(dst, kind="stable")
    seg_start = np.zeros(N_HIGH + 1, np.int64)
    np.cumsum(counts, out=seg_start[1:])

    # split hyperedge ids into 8 shards balanced by edge count
    target = E / N_CORES
    bounds = [0]
    for c in range(1, N_CORES):
        bounds.append(int(np.searchsorted(seg_start, round(target * c))))
    bounds.append(N_HIGH)

    cores = []
    for c in range(N_CORES):
        s_lo, s_hi = bounds[c], bounds[c + 1]
        segs = np.arange(s_lo, s_hi)
        cnt = counts[s_lo:s_hi]
        assert len(segs) <= SEG_CAP, (c, len(segs))
        assert cnt.max(initial=0) <= EDGE_CAP

        # bin-pack segments into NGROUP groups (<=128 segs, <=EDGE_CAP edges)
        g_load = np.zeros(NGROUP, np.int64)
        g_segs = [[] for _ in range(NGROUP)]
        for s in segs[np.argsort(-cnt, kind="stable")]:
            k = counts[s]
            ok = [g for g in range(NGROUP)
                  if len(g_segs[g]) < 128 and g_load[g] + k <= EDGE_CAP]
            assert ok, "bin packing failed; raise TPG/NGROUP"
            g = min(ok, key=lambda g: g_load[g])
            g_segs[g].append(s)
            g_load[g] += k

        edge_orig = np.full((NGROUP, TPG * 128), -1, np.int64)
        gseg = np.full(NGROUP * 128, -1, np.int64)
        for g in range(NGROUP):
            j = 0
            for ig, s in enumerate(g_segs[g]):
                gseg[g * 128 + ig] = s
                e_ids = order[seg_start[s]:seg_start[s + 1]]
                n = len(e_ids)
                edge_orig[g, j:j + n] = e_ids
                j += n
            # slot j -> tile k=j//128, partition p=j%128
        cores.append(dict(g_segs=g_segs, edge_orig=edge_orig,
                          valid=edge_orig >= 0, gseg=gseg))
    return cores, order, seg_start


def _prep(src, dst):
    cores, order, seg_start = _prep_edges(dst)
    counts = np.bincount(dst, minlength=N_HIGH).astype(np.int64)
    for cd in cores:
        eo = cd["edge_orig"]          # [NGROUP, TPG*128] slot-order
        valid = cd["valid"]
        # slot j in group: tile k=j//128, partition p=j%128
        src_slot = np.where(valid, src[np.clip(eo, 0, None)], 0)
        # dstig already filled per-slot during packing -> redo cleanly here
        srcg = src_slot.reshape(NGROUP, TPG, 128).transpose(0, 2, 1)
        cd["srcg"] = np.ascontiguousarray(srcg.astype(np.int32))
        dig_slot = np.full((NGROUP, TPG * 128), 999.0, np.float32)
        # in-group id per slot: recompute from g_segs
        for g, segs_g in enumerate(cd["g_segs"]):
            j = 0
            for ig, s in enumerate(segs_g):
                n = int(counts[s])
                dig_slot[g, j:j + n] = ig
                j += n
        import ml_dtypes
        cd["dstig"] = np.ascontiguousarray(
            dig_slot.reshape(NGROUP, TPG, 128).transpose(0, 2, 1)).astype(
                ml_dtypes.bfloat16)
    return cores


def _run(nc, in_maps):
    _maybe_install_profile_hook()
    res = run_bass_kernel_spmd(nc, in_maps, core_ids=list(range(N_CORES)),
                               trace=_trace_on())
    if res.exec_time_ns is not None:
        _EXEC_TIMES.append(res.exec_time_ns)
    return res.results


def kernel(n_features, W_src, W_dst, att_src, att_dst, W_high, b_high,
           src_idx, dst_idx):
    n_features = np.asarray(n_features, np.float32)
    W_src = np.asarray(W_src, np.float32)
    W_dst = np.asarray(W_dst, np.float32)
    att_src = np.asarray(att_src, np.float32)
    att_dst = np.asarray(att_dst, np.float32)
    W_high = np.asarray(W_high, np.float32)
    b_high = np.asarray(b_high, np.float32)
    src = np.asarray(src_idx).astype(np.int64)
    dst = np.asarray(dst_idx).astype(np.int64)
    E = src.shape[0]

    # fused weights (tiny O(D^2) host math)
    A_dst = np.einsum("dhk,hk->dh", W_src.reshape(D, HEADS, D),
                      att_dst[0]).astype(np.float32)
    A_src = np.einsum("dhk,hk->dh", W_dst.reshape(D, HEADS, D),
                      att_src[0]).astype(np.float32)
    Mq = (0.25 * np.einsum("dhk,ke->dhe", W_src.reshape(D, HEADS, D),
                           W_high)).reshape(D, HEADS * D).astype(np.float32)

    # ---- run 1: per-node scores ----
    nc1 = _build_run1()
    nfT = np.ascontiguousarray(n_features.T)           # [D, N]
    a8 = np.concatenate([A_dst, A_src], axis=1)        # [D, 8]
    in1 = [{"nfT": np.ascontiguousarray(nfT[:, c * NSLICE:(c + 1) * NSLICE]),
            "a8": a8} for c in range(N_CORES)]
    r1 = _run(nc1, in1)
    s_all = np.concatenate([r1[c]["sallT"] for c in range(N_CORES)],
                           axis=1).T                   # [N, 8]

    import ml_dtypes
    cat = np.empty((N_NODES, CAT), ml_dtypes.bfloat16)
    cat[:, :D] = n_features
    cat[:, D:D + 2 * HEADS] = s_all
    cat[:, D + 2 * HEADS] = 1.0

    # ---- host prep of edge structure ----
    cores = _prep(src, dst)

    import ml_dtypes
    iota = np.ascontiguousarray(np.broadcast_to(
        np.arange(128, dtype=np.float32), (128, 128))).astype(ml_dtypes.bfloat16)
    bias = np.ascontiguousarray(np.broadcast_to(b_high, (128, D))).astype(
        np.float32)

    nc2 = _build_run2()
    in2 = [{"cat": cat, "srcg": cd["srcg"], "dstig": cd["dstig"],
            "iota": iota, "bias": bias, "mq": Mq.astype(ml_dtypes.bfloat16)} for cd in cores]
    r2 = _run(nc2, in2)

    # ---- unshard ----
    h_features = np.zeros((N_HIGH, D), np.float32)
    alpha = np.zeros((E, HEADS), np.float32)
    for c, cd in enumerate(cores):
        hf = r2[c]["hf"]                               # [NGROUP*128, D]
        gseg = cd["gseg"]
        m = gseg >= 0
        h_features[gseg[m]] = hf[m]
        al = r2[c]["alpha"].reshape(NGROUP, 128, TPG, HEADS)
        al = al.transpose(0, 2, 1, 3).reshape(NGROUP, TPG * 128, HEADS)
        eo = cd["edge_orig"]
        v = cd["valid"]
        alpha[eo[v]] = al[v]
    return h_features, alpha
